# revision 2
# baseline (speedup 1.0000x reference)
"""CrossAttention Trainium2 kernel, v3.

Same contract/layout as the baseline kernel, but the V projection is
computed transposed ([e, t], W stationary — 128 matmuls instead of 512)
and rotated to natural [t, e] layout with 64 PE transposes.
"""

import numpy as np
import ml_dtypes

import concourse.bass as bass
import concourse.mybir as mybir
import concourse.tile as tile
from concourse import bacc, masks
from concourse.bass_utils import run_bass_kernel_spmd
from concourse.tile import TileContext, ScopedClock

BF16 = mybir.dt.bfloat16
F32 = mybir.dt.float32

B, T, HIN, H, E = 4, 2048, 1024, 16, 64
NCORES = 8
HL = H // NCORES          # heads per core = 2
ES = HL * E               # 128 (e-shard width)
BT = B * T                # 8192
KI = HIN // 128           # 8 contraction k-tiles for projections
NBLK = BT // 128          # 64 t-blocks of 128
SCALE = float(E) ** -0.25

_EXP = mybir.ActivationFunctionType.Exp


def _patch_tail_drain():
    """walrus in this container allows only ONE sync-wait per instruction;
    Tile's kernel-tail drain accumulates one wait per live proc. Spread the
    waits across single-wait NOPs."""
    if getattr(TileContext, "_tail_drain_patched", False):
        return

    def _drain_and_barrier(self, tick_clock, wait_clock):
        probe = self.nc.sync.nop(nofuse=True, hint="tail_wait_probe")
        wait_clock.add_sem_waits(
            probe.ins, ScopedClock({None: tick_clock.global_clock})
        )
        si = probe.ins.sync_info
        waits = list(si.on_wait) if si is not None else []
        if len(waits) > 1:
            probe.ins.sync_info = mybir.SyncInfo(
                on_wait=waits[:1], on_update=list(si.on_update)
            )
            for i in range(1, len(waits)):
                n2 = self.nc.sync.nop(nofuse=True, hint=f"tail_wait_{i}")
                n2.ins.sync_info = mybir.SyncInfo(on_wait=[waits[i]], on_update=[])
        self.nc.sync.drain()
        self.nc.all_engine_barrier()
        popped = self.nc._tile_sem_poison_stack.pop()
        assert popped is self._sem_poison
        self.nc.clear_and_free_semaphores(list(self.sems.allocated().values()))
        self.nc.all_engine_barrier()

    TileContext._drain_and_barrier = _drain_and_barrier
    TileContext._tail_drain_patched = True


def emit_body(nc, tc, xq, xkv, wq, wk, wv, out):
    with tc.tile_pool(name="persist", bufs=1) as persist:
        # persistent SBUF tensors for the attention phase
        qT_s = persist.tile([128, BT], BF16)          # [e_shard, b*t]
        kT_s = persist.tile([128, BT], BF16)          # [e_shard, b*t]
        vT_s = persist.tile([128, BT], BF16)          # [e_shard, b*t]
        # v in natural layout + interleaved ones column per head:
        # per 128-t-block: [h0 e(64) | 1 | h1 e(64) | 1] -> 130 cols
        v_sb = persist.tile([128, NBLK, 2 * (E + 1)], BF16)
        nc.vector.memset(v_sb, 1.0)
        ident = persist.tile([128, 128], BF16)
        masks.make_identity(nc, ident)

        # ---------------- Phase 1: projections ----------------
        with tc.tile_pool(name="xin", bufs=2) as xin, \
             tc.tile_pool(name="win", bufs=1) as win, \
             tc.tile_pool(name="ps_q", bufs=2, space="PSUM") as ps_q, \
             tc.tile_pool(name="ps_k", bufs=2, space="PSUM") as ps_k, \
             tc.tile_pool(name="ps_v", bufs=2, space="PSUM") as ps_v:
            wq_sb = win.tile([128, KI, ES], BF16, tag="wq")
            wk_sb = win.tile([128, KI, ES], BF16, tag="wk")
            wv_sb = win.tile([128, KI, ES], BF16, tag="wv")
            for i in range(KI):
                nc.sync.dma_start(out=wq_sb[:, i, :], in_=wq[i, :, :])
                nc.sync.dma_start(out=wk_sb[:, i, :], in_=wk[i, :, :])
                nc.sync.dma_start(out=wv_sb[:, i, :], in_=wv[i, :, :])

            NCH = BT // 512  # 16 chunks of 512 t-columns
            for nb in range(NCH):
                cs = slice(nb * 512, (nb + 1) * 512)
                xq_sb = xin.tile([128, KI, 512], BF16, tag="xq")
                xkv_sb = xin.tile([128, KI, 512], BF16, tag="xkv")
                for i in range(KI):
                    nc.sync.dma_start(out=xq_sb[:, i, :], in_=xq[i, :, cs])
                    nc.sync.dma_start(out=xkv_sb[:, i, :], in_=xkv[i, :, cs])

                # qT/kT/vT [e, t]: lhsT = W[i-block, e], rhs = xT[i-block, t]
                psq = ps_q.tile([128, 512], F32)
                psk = ps_k.tile([128, 512], F32)
                psv = ps_v.tile([128, 512], F32)
                for i in range(KI):
                    nc.tensor.matmul(psq, wq_sb[:, i, :], xq_sb[:, i, :],
                                     start=(i == 0), stop=(i == KI - 1))
                for i in range(KI):
                    nc.tensor.matmul(psk, wk_sb[:, i, :], xkv_sb[:, i, :],
                                     start=(i == 0), stop=(i == KI - 1))
                for i in range(KI):
                    nc.tensor.matmul(psv, wv_sb[:, i, :], xkv_sb[:, i, :],
                                     start=(i == 0), stop=(i == KI - 1))
                nc.vector.tensor_copy(qT_s[:, cs], psq)
                nc.vector.tensor_copy(kT_s[:, cs], psk)
                nc.vector.tensor_copy(vT_s[:, cs], psv)

        # rotate vT -> natural v blocks via PE transpose
        with tc.tile_pool(name="ptr", bufs=2, space="PSUM") as ptr:
            for blk in range(NBLK):
                tr = ptr.tile([128, 128], BF16, tag="tr")
                nc.tensor.transpose(
                    tr, vT_s[:, blk * 128:(blk + 1) * 128], ident)
                dst = v_sb[:, blk, :].rearrange(
                    "p (h e1) -> p h e1", h=HL)[:, :, :E]
                src = tr.rearrange("p (h e) -> p h e", h=HL)
                nc.vector.tensor_copy(dst, src)

        # ---------------- Phase 2: attention ----------------
        QC = 1024                 # q-chunk width
        NQC = T // QC             # 2 chunks per batch
        KT = T // 128             # 16 k-tiles per batch
        with tc.tile_pool(name="pP", bufs=6) as pP, \
             tc.tile_pool(name="pp_x", bufs=1, space="PSUM") as pp_x, \
             tc.tile_pool(name="pp_o", bufs=2, space="PSUM") as pp_o, \
             tc.tile_pool(name="dn", bufs=2) as dn, \
             tc.tile_pool(name="dr", bufs=2, space="DRAM") as dr, \
             tc.tile_pool(name="po", bufs=2) as po:
            for b in range(B):
                for qc in range(NQC):
                    q0 = b * T + qc * QC  # column base into qT_s
                    X = pp_x.tile([128, 2, QC], F32)       # S^T staging (4 banks)
                    oT = [pp_o.tile([E + 1, QC], F32, tag="ot", name=f"oT{h}")
                          for h in range(HL)]
                    P_tiles = [[None] * HL for _ in range(KT)]
                    for kt in range(KT):
                        k0 = b * T + kt * 128
                        blk = b * KT + kt
                        for h in range(HL):
                            s = h  # X half
                            hp = slice(h * E, (h + 1) * E)
                            # S^T[k_tile, q_chunk] = K^T.T @ Q^T (contraction
                            # e=64, two heads row-packed on PE rows 0:64/64:128)
                            for ns in range(QC // 512):
                                nc.tensor.matmul(
                                    X[:, s, ns * 512:(ns + 1) * 512],
                                    kT_s[hp, k0:k0 + 128],
                                    qT_s[hp, q0 + ns * 512: q0 + (ns + 1) * 512],
                                    start=True, stop=True)
                            # P = exp(S^T) -> bf16
                            Pt = pP.tile([128, QC], BF16, tag="P")
                            P_tiles[kt][h] = Pt
                            nc.scalar.activation(out=Pt, in_=X[:, s, :], func=_EXP)
                        for h in range(HL):
                            # oT[e|den, q] += [v|1].T @ P  (contraction k=128)
                            vp = v_sb[:, blk, h * (E + 1): (h + 1) * (E + 1)]
                            Pt = P_tiles[kt][h]
                            for ns in range(QC // 512):
                                nc.tensor.matmul(
                                    oT[h][:, ns * 512:(ns + 1) * 512],
                                    vp,
                                    Pt[:, ns * 512:(ns + 1) * 512],
                                    start=(kt == 0), stop=(kt == KT - 1))
                    # ---- normalize: out = oT[0:E] / oT[E] ----
                    # r = 1/denominator, computed at partition 64 (no partition
                    # shift), bounced through DRAM for the partition-broadcast
                    rscr = dr.tile([HL, QC], F32, tag="rscr")
                    for h in range(HL):
                        dcp = dn.tile([E + 1, QC], F32, tag="d", name=f"dcp{h}")
                        nc.vector.tensor_copy(dcp[E:E + 1, :], oT[h][E:E + 1, :])
                        nc.sync.dma_start(out=rscr[h, :], in_=dcp[E:E + 1, :])
                    for h in range(HL):
                        drep = dn.tile([E, QC], F32, tag="drep", name=f"drep{h}")
                        nc.sync.dma_start(
                            out=drep, in_=rscr[h, :].partition_broadcast(E))
                        rrep = dn.tile([E, QC], F32, tag="rrep", name=f"rrep{h}")
                        nc.vector.reciprocal_approx_fast(out=rrep, in_=drep)
                        osb = po.tile([E, QC], F32, tag="osb")
                        nc.vector.tensor_mul(osb, oT[h][:E, :], rrep)
                        nc.sync.dma_start(
                            out=out[h * E:(h + 1) * E, q0:q0 + QC], in_=osb)


def build_nc(reps: int = 1, phase1_only=False):
    _patch_tail_drain()
    nc = bacc.Bacc(None)
    xq = nc.declare_dram_parameter("xq", [KI, 128, BT], BF16, isOutput=False)
    xkv = nc.declare_dram_parameter("xkv", [KI, 128, BT], BF16, isOutput=False)
    wq = nc.declare_dram_parameter("wq", [KI, 128, ES], BF16, isOutput=False)
    wk = nc.declare_dram_parameter("wk", [KI, 128, ES], BF16, isOutput=False)
    wv = nc.declare_dram_parameter("wv", [KI, 128, ES], BF16, isOutput=False)
    out = nc.declare_dram_parameter("out", [ES, BT], F32, isOutput=True)
    with TileContext(nc) as tc:
        for _ in range(reps):
            emit_body(nc, tc, xq, xkv, wq, wk, wv, out)
    nc.finalize()
    return nc


def make_in_maps(query, key_value, Wq, Wk, Wv):
    """Host-side sharding/layout. Returns per-core input maps."""
    bf = ml_dtypes.bfloat16
    xq = np.ascontiguousarray(
        query.reshape(BT, HIN).T).astype(bf).reshape(KI, 128, BT)
    xkv = np.ascontiguousarray(
        key_value.reshape(BT, HIN).T).astype(bf).reshape(KI, 128, BT)
    wq_s = (Wq.astype(np.float32) * SCALE).astype(bf)
    wk_s = (Wk.astype(np.float32) * SCALE).astype(bf)
    wv_s = Wv.astype(bf)
    in_maps = []
    for c in range(NCORES):
        cols = slice(c * ES, (c + 1) * ES)
        in_maps.append({
            "xq": xq,
            "xkv": xkv,
            "wq": np.ascontiguousarray(wq_s[:, cols]).reshape(KI, 128, ES),
            "wk": np.ascontiguousarray(wk_s[:, cols]).reshape(KI, 128, ES),
            "wv": np.ascontiguousarray(wv_s[:, cols]).reshape(KI, 128, ES),
        })
    return in_maps


def assemble_output(results):
    full = np.concatenate(
        [np.asarray(results[c]["out"]) for c in range(NCORES)], axis=0)
    return np.ascontiguousarray(full.T).reshape(B, T, H * E).astype(np.float32)


_NC_CACHE = {}


def kernel(query, key_value, Wq, Wk, Wv):
    query = np.asarray(query, dtype=np.float32)
    key_value = np.asarray(key_value, dtype=np.float32)
    Wq = np.asarray(Wq, dtype=np.float32)
    Wk = np.asarray(Wk, dtype=np.float32)
    Wv = np.asarray(Wv, dtype=np.float32)

    if "nc" not in _NC_CACHE:
        _NC_CACHE["nc"] = build_nc(reps=1)
    nc = _NC_CACHE["nc"]
    in_maps = make_in_maps(query, key_value, Wq, Wk, Wv)
    res = run_bass_kernel_spmd(nc, in_maps, list(range(NCORES)))
    return assemble_output(res.results)


if __name__ == "__main__":
    rng = np.random.default_rng(0)
    q = rng.standard_normal((B, T, HIN), dtype=np.float32)
    kv = rng.standard_normal((B, T, HIN), dtype=np.float32)
    s = 1.0 / np.sqrt(HIN)
    wq = rng.uniform(-s, s, (HIN, H * E)).astype(np.float32)
    wk = rng.uniform(-s, s, (HIN, H * E)).astype(np.float32)
    wv = rng.uniform(-s, s, (HIN, H * E)).astype(np.float32)
    out = kernel(query=q, key_value=kv, Wq=wq, Wk=wk, Wv=wv)
    print("out", out.shape, out.dtype, np.abs(out).mean())
